# revision 16
# baseline (speedup 1.0000x reference)
"""GCN message-passing + dense sigmoid(h @ S @ h.T) kernel for 8 TRN2 NeuronCores.

Strategy (SPMD, one NEFF on cores 0-7):
  - Nodes row-sharded: core k owns rows [1250k, 1250(k+1)), as 10 blocks of 125.
  - Per layer: every core computes the full t = h @ W table (row-major, bf16,
    zero-padded to 128 cols) into its HBM; edge messages are fetched with
    dma_gather (256B rows); SpMM is a PSUM-accumulated matmul against
    host-built one-hot-times-val "P" chunks (edge -> local row), giving the
    transposed feature block h_T directly; ELU is composed from
    relu(x) + exp(min(x,0)) - 1.
  - h shards are exchanged with an AllGather collective between layers.
  - Final phase: hS_T = S.T @ h3_shard_T, then out rows = sigmoid(hS_block.T
    @ h3_T_full) streamed out as f32.

Numerics: bf16 tables/messages/weights with f32 PSUM accumulation. The
architecture saturates the final sigmoid (min logit ~27 for this input
family), so bf16 is far inside tolerance.
"""

import sys

if "/opt/trn_rl_repo" not in sys.path:
    sys.path.insert(0, "/opt/trn_rl_repo")

import numpy as np
import ml_dtypes

N = 10000
E = 320000
D = 128
DOUT = 64
NCORES = 8
RPC = N // NCORES          # rows per core = 1250
BLK = 125                  # spmm output block (psum partition dim)
NBLK = RPC // BLK          # 10 blocks per core
GBLK = 80                  # global 125-row blocks
TBLK = 79                  # 128-row blocks of the t table
TPAD = TBLK * 128          # t table rows incl. padding = 10112

_CACHE = {}
LAST_RESULTS = None        # BassKernelResults of the most recent run (for test.py)


def _build(cmax: int, stage: int = 7):
    """Build + compile the SPMD bass program. Cached per (cmax, stage).

    stage: 1=loads+x.T  2=+t1 table  3=+layer0 spmm  4=+AG0  5=+layers1-2
           6=+hS  7=full
    """
    key = (cmax, stage)
    if key in _CACHE:
        return _CACHE[key]

    import concourse.bass as bass  # noqa: F401
    import concourse.mybir as mybir
    import concourse.tile as tile
    from concourse import bacc
    from concourse.masks import make_identity

    bf16 = mybir.dt.bfloat16
    f32 = mybir.dt.float32
    i16 = mybir.dt.int16
    AF = mybir.ActivationFunctionType
    ALU = mybir.AluOpType

    SLOTS = cmax * 128

    nc = bacc.Bacc(
        "TRN2", target_bir_lowering=False, debug=False, num_devices=NCORES
    )

    x_in = nc.dram_tensor("x", [N, D], f32, kind="ExternalInput")
    P_in = nc.dram_tensor("P", [128, NBLK * SLOTS], bf16, kind="ExternalInput")
    idx_in = nc.dram_tensor("idx", [128, NBLK * SLOTS // 16], i16, kind="ExternalInput")
    w_ins = [
        nc.dram_tensor(f"W{i}s", [D, D], bf16, kind="ExternalInput") for i in range(3)
    ]
    s_in = nc.dram_tensor("Ssym", [DOUT, DOUT], bf16, kind="ExternalInput")
    out_t = nc.dram_tensor("out", [RPC, N], f32, kind="ExternalOutput")

    with tile.TileContext(nc) as tc:
        with (
            tc.tile_pool(name="const", bufs=1) as pconst,
            tc.tile_pool(name="big", bufs=1) as pbig,
            tc.tile_pool(name="xload", bufs=3) as pxl,
            tc.tile_pool(name="tstage", bufs=3) as pstage,
            tc.tile_pool(name="msgs", bufs=2) as pmsgs,
            tc.tile_pool(name="elu", bufs=2) as pelu,
            tc.tile_pool(name="outp", bufs=2) as pout,
            tc.tile_pool(name="psA", bufs=2, space="PSUM") as psA,
            tc.tile_pool(name="psB", bufs=2, space="PSUM") as psB,
            tc.tile_pool(name="psHS", bufs=1, space="PSUM") as psHS,
            tc.tile_pool(name="psO", bufs=3, space="PSUM") as psO,
            tc.tile_pool(name="dram", bufs=1, space="DRAM") as pdram,
        ):
            ident = pconst.tile([128, 128], f32, name="ident")
            make_identity(nc, ident[:])

            w_sb = []
            for i in range(3):
                w = pconst.tile([D, D], bf16, name=f"w{i}sb")
                nc.sync.dma_start(out=w[:], in_=w_ins[i].ap())
                w_sb.append(w)
            s_sb = pconst.tile([DOUT, DOUT], bf16, name="ssb")
            nc.sync.dma_start(out=s_sb[:], in_=s_in.ap())

            P_sb = pbig.tile([128, NBLK * SLOTS], bf16, name="Psb")
            nc.sync.dma_start(out=P_sb[:], in_=P_in.ap())
            idx_sb = pbig.tile([128, NBLK * SLOTS // 16], i16, name="idxsb")
            nc.sync.dma_start(out=idx_sb[:], in_=idx_in.ap())

            hT = pbig.tile([128, TPAD], bf16, name="hT")
            nc.gpsimd.memset(hT[:, N:TPAD], 0.0)
            h3T = pbig.tile([DOUT, N], bf16, name="h3T")
            hS = pbig.tile([DOUT, RPC], bf16, name="hS")
            hsh = [pbig.tile([128, RPC], bf16, name=f"hsh{l}") for l in range(3)]

            # ---- x.T -> hT (layer-0 features, transposed via PE) ----
            # batched loads: 8 row-blocks per DMA (full blocks only; the
            # 16-row tail block is loaded separately)
            xr = x_in.ap()
            FULLB = N // 128  # 78 full 128-row blocks
            for g0 in range(0, FULLB, 8):
                gn = min(8, FULLB - g0)
                x_sb = pxl.tile([128, 8 * D], f32, tag="xld")
                nc.sync.dma_start(
                    out=x_sb[:, : gn * D].rearrange("p (g j) -> p g j", j=D),
                    in_=xr[g0 * 128 : (g0 + gn) * 128, :].rearrange(
                        "(g p) j -> p g j", p=128
                    ),
                )
                for g in range(gn):
                    ps = psB.tile([128, 128], f32, tag="psB")
                    nc.tensor.transpose(
                        ps[:], x_sb[:, g * D : (g + 1) * D], ident[:]
                    )
                    nc.vector.tensor_copy(
                        out=hT[:, (g0 + g) * 128 : (g0 + g + 1) * 128], in_=ps[:]
                    )
            # tail: rows 9984..10000
            rs = N - FULLB * 128
            x_sb = pxl.tile([128, 8 * D], f32, tag="xld")
            nc.sync.dma_start(out=x_sb[:rs, :D], in_=xr[FULLB * 128 : N, :])
            ps = psB.tile([128, 128], f32, tag="psB")
            nc.tensor.transpose(ps[:, :rs], x_sb[:rs, :D], ident[:rs, :rs])
            nc.vector.tensor_copy(out=hT[:, FULLB * 128 : N], in_=ps[:, :rs])

            ttabs = [pdram.tile([TPAD, D], bf16, name=f"ttab{l}") for l in range(3)]
            agin = [pdram.tile([128, RPC], bf16, name=f"agin{l}") for l in range(2)]
            agout = [
                pdram.tile(
                    [NCORES, 128, RPC], bf16, addr_space="Shared", name=f"agout{l}"
                )
                for l in range(2)
            ]
            agin3 = pdram.tile([DOUT, RPC], bf16, name="agin3")
            agout3 = pdram.tile(
                [NCORES, DOUT, RPC], bf16, addr_space="Shared", name="agout3"
            )

            rg = [list(range(NCORES))]

            nlayers = 0 if stage < 2 else (1 if stage < 5 else 3)
            for l in range(nlayers):
                ttab = ttabs[l]
                # t table: t = h @ W, row-major, 8 128-row blocks per DMA
                for g0 in range(0, TBLK, 8):
                    gn = min(8, TBLK - g0)
                    tst = pstage.tile([128, 8 * D], bf16, tag="tst")
                    for g in range(gn):
                        gb = g0 + g
                        ps = psB.tile([128, 128], f32, tag="psB")
                        nc.tensor.matmul(
                            ps[:],
                            lhsT=hT[:, gb * 128 : (gb + 1) * 128],
                            rhs=w_sb[l][:],
                            start=True,
                            stop=True,
                        )
                        nc.vector.tensor_copy(
                            out=tst[:, g * D : (g + 1) * D], in_=ps[:]
                        )
                    nc.sync.dma_start(
                        out=ttab[g0 * 128 : (g0 + gn) * 128, :].rearrange(
                            "(g p) j -> p g j", p=128
                        ),
                        in_=tst[:, : gn * D].rearrange("p (g j) -> p g j", j=D),
                    )

                # spmm per 125-row block
                for b in range(NBLK if stage >= 3 else 0):
                    msgs = pmsgs.tile([128, SLOTS], bf16, tag="msgs")
                    # dma_gather with single_packet is capped at 64 descs x
                    # 16 engines = 1024 indices per call
                    for q0 in range(0, SLOTS, 1024):
                        qn = min(1024, SLOTS - q0)
                        nc.gpsimd.dma_gather(
                            out_ap=msgs[:, q0 : q0 + qn].rearrange(
                                "p (c e) -> p c e", e=D
                            ),
                            in_ap=ttab[:],
                            idxs_ap=idx_sb[
                                :,
                                (b * SLOTS + q0) // 16 : (b * SLOTS + q0 + qn) // 16,
                            ],
                            num_idxs=qn,
                            num_idxs_reg=qn,
                            elem_size=D,
                        )
                    ps = psA.tile([128, 128], f32, tag="psA")
                    for c in range(cmax):
                        nc.tensor.matmul(
                            ps[:],
                            lhsT=msgs[:, c * 128 : (c + 1) * 128],
                            rhs=P_sb[:, (b * cmax + c) * 128 : (b * cmax + c + 1) * 128],
                            start=(c == 0),
                            stop=(c == cmax - 1),
                        )
                    # ELU(ps) = relu(ps) + exp(min(ps,0)) - 1
                    m_sb = pelu.tile([128, BLK], f32, tag="elu_m")
                    nc.vector.tensor_scalar_min(m_sb[:], ps[:, :BLK], 0.0)
                    e_sb = pelu.tile([128, BLK], f32, tag="elu_e")
                    nc.scalar.activation(e_sb[:], m_sb[:], AF.Exp)
                    r_sb = pelu.tile([128, BLK], f32, tag="elu_r")
                    nc.scalar.activation(r_sb[:], ps[:, :BLK], AF.Relu)
                    a_sb = pelu.tile([128, BLK], f32, tag="elu_a")
                    nc.vector.tensor_tensor(
                        out=a_sb[:], in0=e_sb[:], in1=r_sb[:], op=ALU.add
                    )
                    nc.vector.tensor_scalar_add(
                        hsh[l][:, b * BLK : (b + 1) * BLK], a_sb[:], -1.0
                    )

                if stage < 4:
                    continue
                if l < 2:
                    nc.sync.dma_start(out=agin[l][:], in_=hsh[l][:])
                    nc.gpsimd.collective_compute(
                        "AllGather",
                        ALU.bypass,
                        replica_groups=rg,
                        ins=[agin[l][:]],
                        outs=[agout[l][:]],
                    )
                    nc.sync.dma_start(
                        out=hT[:, :N].rearrange("p (r c) -> p r c", r=NCORES),
                        in_=agout[l][:].rearrange("r p c -> p r c"),
                    )
                else:
                    nc.sync.dma_start(out=agin3[:], in_=hsh[l][:DOUT, :])
                    nc.gpsimd.collective_compute(
                        "AllGather",
                        ALU.bypass,
                        replica_groups=rg,
                        ins=[agin3[:]],
                        outs=[agout3[:]],
                    )
                    nc.sync.dma_start(
                        out=h3T[:].rearrange("p (r c) -> p r c", r=NCORES),
                        in_=agout3[:].rearrange("r p c -> p r c"),
                    )

            # hS_T = S.T @ h3_shard_T   (shard lives in hsh[2][:64])
            for off, w in ((0, 500), (500, 500), (1000, 250)) if stage >= 6 else ():
                ps = psHS.tile([DOUT, 500], f32, tag="psHS")
                nc.tensor.matmul(
                    ps[:, :w],
                    lhsT=s_sb[:],
                    rhs=hsh[2][:DOUT, off : off + w],
                    start=True,
                    stop=True,
                )
                nc.vector.tensor_copy(out=hS[:, off : off + w], in_=ps[:, :w])

            # final: out rows = sigmoid(hS_block.T @ h3T), streamed per block
            CW = 500            # matmul free dim
            PIECE = 2500        # per-DMA column chunk
            import os as _os

            finblk = int(_os.environ.get("GCN_FINBLK", str(NBLK)))
            for b in range(min(NBLK, finblk) if stage >= 7 else 0):
                for j in range(N // PIECE):
                    outp = pout.tile([BLK, PIECE], f32, tag="outp")
                    for cc in range(PIECE // CW):
                        nch = j * (PIECE // CW) + cc
                        ps = psO.tile([BLK, CW], f32, tag="psO")
                        nc.tensor.matmul(
                            ps[:],
                            lhsT=hS[:, b * BLK : (b + 1) * BLK],
                            rhs=h3T[:, nch * CW : (nch + 1) * CW],
                            start=True,
                            stop=True,
                        )
                        nc.scalar.activation(
                            outp[:, cc * CW : (cc + 1) * CW], ps[:], AF.Sigmoid
                        )
                    if b < NBLK - 1 or _os.environ.get("GCN_SKIP_LAST_DMA", "0") != "1":
                        nc.sync.dma_start(
                            out=out_t.ap()[
                                b * BLK : (b + 1) * BLK, j * PIECE : (j + 1) * PIECE
                            ],
                            in_=outp[:],
                        )

    nc.compile()
    _CACHE[key] = nc
    return nc


def _prepare(x, edge_row, edge_col, edge_val, W0, W1, W2, Wb):
    """Host preprocessing: edge sort/pad, P one-hot*val chunks, wrapped idxs."""
    bf = ml_dtypes.bfloat16
    g = edge_row // BLK                      # global 125-row block per edge
    order = np.argsort(g, kind="stable")
    go = g[order]
    co = edge_col[order].astype(np.int64)
    vo = edge_val[order]
    ro = edge_row[order].astype(np.int64)
    cnt = np.bincount(g, minlength=GBLK)
    starts = np.concatenate([[0], np.cumsum(cnt)])
    slot = np.arange(E) - starts[go]
    cmax = int(np.ceil(cnt.max() / 128))
    SLOTS = cmax * 128

    colp = np.zeros((GBLK, SLOTS), np.int16)
    colp[go, slot] = co.astype(np.int16)
    Pfull = np.zeros((GBLK, cmax, 128, 128), bf)
    Pfull[go, slot // 128, slot % 128, ro - go * BLK] = vo.astype(bf)

    # idx wrapped in 16 partitions: element i of a block at [i%16, i//16]
    idx_wrap = colp.reshape(GBLK, SLOTS // 16, 16).transpose(0, 2, 1)  # [G,16,S]

    S_sym = ((Wb + Wb.T) * 0.5).astype(bf)
    W2p = np.zeros((D, D), np.float32)
    W2p[:, :DOUT] = W2
    wlist = [W0.astype(bf), W1.astype(bf), W2p.astype(bf)]

    in_maps = []
    for k in range(NCORES):
        Pk = (
            Pfull[NBLK * k : NBLK * (k + 1)]
            .transpose(2, 0, 1, 3)
            .reshape(128, NBLK * SLOTS)
            .copy()
        )
        idx_k = (
            idx_wrap[NBLK * k : NBLK * (k + 1)]
            .transpose(1, 0, 2)
            .reshape(16, NBLK * SLOTS // 16)
        )
        idx_k = np.tile(idx_k, (8, 1)).copy()  # replicate to 128 partitions
        m = {
            "x": x,
            "P": Pk,
            "idx": idx_k,
            "W0s": wlist[0],
            "W1s": wlist[1],
            "W2s": wlist[2],
            "Ssym": S_sym,
        }
        in_maps.append(m)
    return in_maps, cmax


def kernel(x, edge_row, edge_col, edge_val, W0, W1, W2, Wb):
    global LAST_RESULTS
    x = np.ascontiguousarray(np.asarray(x, np.float32))
    edge_row = np.asarray(edge_row, np.int32)
    edge_col = np.asarray(edge_col, np.int32)
    edge_val = np.asarray(edge_val, np.float32)
    W0 = np.asarray(W0, np.float32)
    W1 = np.asarray(W1, np.float32)
    W2 = np.asarray(W2, np.float32)
    Wb = np.asarray(Wb, np.float32)

    import os

    stage = int(os.environ.get("GCN_STAGE", "7"))
    in_maps, cmax = _prepare(x, edge_row, edge_col, edge_val, W0, W1, W2, Wb)
    nc = _build(cmax, stage)

    from concourse.bass_utils import run_bass_kernel_spmd

    res = run_bass_kernel_spmd(nc, in_maps, core_ids=list(range(NCORES)))
    LAST_RESULTS = res
    return np.concatenate(
        [res.results[k]["out"] for k in range(NCORES)], axis=0
    )
